# revision 48
# baseline (speedup 1.0000x reference)
"""MoE grouped-GEMM (FMoELinear) on 8 trn2 NeuronCores.

Strategy (expert parallelism):
  - 32 experts, 8 cores -> 4 experts per core, chosen by a snake assignment
    over the count-sorted experts so per-core token totals balance.
  - Tokens arrive pre-sorted by expert. Each core's 4 experts map to 4
    fixed-size SEGMENTS (sizes shared by all cores so one SPMD program
    serves every core): segment j holds the core's j-th largest expert,
    and S_j = max over cores of the j-th largest count, rounded to 32.
    This cuts padding from a uniform per-expert cap to ~0.3%.
  - Device computes yt[o, t] = sum_i W[e][o, i] * x[t, i] per segment with
    the weight stationary in the PE array:
        lhsT = wt[i_chunk, j*256 + oc*128 : +128]   (128 x 128, stationary)
        rhs  = xt tile    [i_chunk, token span]     (128 x <=512, moving)
    accumulating the two i-chunks into PSUM, then casting PSUM->SBUF->HBM.
  - Host gathers the non-padded columns back into token order.

Precision/bandwidth plan: rel-err budget is 2e-2; stream x and y as fp8
E3M4 (~1.3% rms quant noise each; measured total 1.75e-2) to halve HBM
traffic and SBUF pressure. x is pre-scaled by XSCALE (folded back via
w/XSCALE) so fewer values land in the E3M4 subnormal range while PSUM y
stays unscaled (|y|max ~8.9 must fit E3M4's +-15.5 on the cast).

Engine layout: PE does 2 matmuls per token span (K=256 split in two
128-row chunks) at 1 cycle/column; the PSUM->SBUF casts alternate between
DVE and ACT. Only three DMA rings exist (sync/scalar HWDGE + gpsimd
SWDGE) at ~165-200 GB/s each, and engine instruction streams are
in-order, so x loads ride the sync ring (which does nothing else), the
weight panels and half the y stores ride the scalar ring, and the other
half of the stores ride the gpsimd ring. The first chunk is split into
512-token pieces and one early chunk is loaded via gpsimd so supply
outruns the PE during queue spin-up; warm-up matmuls cover that window
and pull the PE out of its low power-state. The final chunks store in
512-token pieces alternating scalar/sync rings, anchored so the very
last piece casts on both DVE and ACT in parallel and stores on the
then-idle sync ring (the gpsimd ring drains slowly at teardown).

DRAM layout: x and y are stored chunk-major as [128, sum_j 2*S_j] so every
full-chunk DMA is a single 2*CHUNK-byte contiguous run per partition
(bigger SDMA descriptors -> better per-queue DMA throughput). Block c at
token offset t0 holds tokens [t0, t0+cw) as [2, cw]: row 0 = in-features
0..127, row 1 = in-features 128..255 (for y: out-features).
"""

import os
import sys
import types

import ml_dtypes
import numpy as np

import concourse.bacc as bacc
import concourse.mybir as mybir
import concourse.tile as tile
from concourse.bass_utils import run_bass_kernel_spmd


def _ensure_axon_hooks_importable():
    """bass_utils imports antenv.axon_hooks when tracing is requested; some
    images lack that module. Provide a no-op fallback so a stray BASS_TRACE
    env var can't crash the kernel (tracing then degrades gracefully)."""
    try:
        import antenv  # noqa: F401
    except ImportError:
        return
    try:
        import antenv.axon_hooks  # noqa: F401
    except ImportError:
        mod = types.ModuleType("antenv.axon_hooks")
        holder = [None]
        mod.set_axon_ntff_profile_hook = lambda h: holder.__setitem__(0, h)
        mod.get_axon_ntff_profile_hook = lambda: holder[0]
        sys.modules["antenv.axon_hooks"] = mod
        import antenv as _antenv

        _antenv.axon_hooks = mod


_ensure_axon_hooks_importable()

NCORES = 8
D = 256  # in/out feature dim
EPC = 4  # experts per core
SEG_GRAN = 32  # segment size granularity (tokens)

# observability for test harness
last_exec_time_ns = None
last_results = None

_prog_cache = {}


def _dt1(name):
    if name == "f32":
        return mybir.dt.float32, np.dtype(np.float32)
    if name == "f32r":
        return mybir.dt.float32r, np.dtype(np.float32)
    if name == "f16":
        return mybir.dt.float16, np.dtype(np.float16)
    if name == "bf16":
        return mybir.dt.bfloat16, np.dtype(ml_dtypes.bfloat16)
    if name == "f8e3":
        return mybir.dt.float8e3, np.dtype(ml_dtypes.float8_e3m4)
    if name == "f8e4":
        return mybir.dt.float8e4, np.dtype(ml_dtypes.float8_e4m3)
    if name == "f8e5":
        return mybir.dt.float8e5, np.dtype(ml_dtypes.float8_e5m2)
    raise ValueError(name)


class _Cfg:
    def __init__(self):
        # "xdt" or "xdt+wdt": moving (x) and stationary (w) matmul dtypes
        self.mm_dt = os.environ.get("BASSMOE_MM_DT", "f8e3+f16")
        self.y_dt = os.environ.get("BASSMOE_Y_DT", "f8e3")
        self.xscale = float(os.environ.get("BASSMOE_XSCALE", "2"))
        self.chunk = int(os.environ.get("BASSMOE_CHUNK", "2048"))
        # cast-engine pattern, cycled per PSUM-bank cast: d=DVE, a=ACT
        self.cast_pat = os.environ.get("BASSMOE_CAST_PAT", "da")
        # engines issuing y stores, cycled per chunk: g=gpsimd, a=ACT, s=SP
        self.st_pat = os.environ.get("BASSMOE_ST_PAT", "ga")
        self.xbufs = int(os.environ.get("BASSMOE_XBUFS", "8"))
        self.ybufs = int(os.environ.get("BASSMOE_YBUFS", "6"))
        self.psbufs = int(os.environ.get("BASSMOE_PSBUFS", "8"))
        self.wide_cast = bool(int(os.environ.get("BASSMOE_WIDE_CAST", "0")))
        self.warm_mms = int(os.environ.get("BASSMOE_WARM_MMS", "6"))
        self.warm_inter = int(os.environ.get("BASSMOE_WARM_INTER", "0"))
        parts = self.mm_dt.split("+")
        self.dt_x, self.np_x = _dt1(parts[0])
        self.dt_w, self.np_w = _dt1(parts[-1])
        self.dt_y, self.np_y = _dt1(self.y_dt)

    def key(self, segs):
        return (
            segs,
            self.mm_dt,
            self.y_dt,
            self.chunk,
            self.cast_pat,
            self.st_pat,
            self.xbufs,
            self.ybufs,
            self.psbufs,
            self.warm_mms,
            self.warm_inter,
            self.wide_cast,
        )


def _chunk_offsets(
    seg: int, chunk: int, first_split: bool = False, last_split: bool = False
):
    """(offset, width) chunks covering [0, seg), width <= chunk.

    first_split breaks the leading chunk into small pieces so the very
    first matmul can start as soon as a short prefix of x has landed;
    last_split tapers the trailing chunks the same way so the final store
    transfers (which serialize after the last casts) are short."""
    out = []
    off = 0
    while off < seg:
        w = min(chunk, seg - off)
        if first_split and off == 0:
            # a 512-token leading piece lands ~0.7us before a 1024 one
            # (the first real matmul waits on the FULL piece); keep the
            # trigger count at three so the serial ~0.7us DMA-trigger
            # instructions don't delay the following chunk loads
            s = 0
            for p in (512, 512, 1024):
                if s >= w:
                    break
                out.append((off + s, min(p, w - s)))
                s += p
            while s < w:
                out.append((off + s, min(1024, w - s)))
                s += 1024
        elif last_split and off + 2 * chunk >= seg:
            s = 0
            while s < w:
                r = w - s
                if off + w == seg and r <= 512:
                    # the very last piece: split small so the final
                    # cast->store chain after the last matmul is short
                    if r > 256:
                        out.append((off + s, r - 192))
                        s += r - 192
                        r = 192
                    out.append((off + s, r))
                    s += r
                else:
                    out.append((off + s, min(512, r)))
                    s += 512
        else:
            out.append((off, w))
        off += w
    return out


def _splits(width: int):
    """(offset, width) matmul spans <= 512 covering [0, width)."""
    out = []
    off = 0
    while off < width:
        w = min(512, width - off)
        out.append((off, w))
        off += w
    return out


def _build_program(cfg: _Cfg, segs: tuple):
    """Build the SPMD Bass program for per-segment token sizes `segs`."""
    width = sum(segs)
    CHUNK = cfg.chunk

    nc = bacc.Bacc(
        "TRN2",
        target_bir_lowering=False,
        debug=False,
        enable_asserts=False,
        num_devices=NCORES,
    )
    # chunk-major layout: [128, wpad + 2*width]; the first wpad columns hold
    # the raw bytes of the four weight panels so they ride ahead of the
    # token stream (bitcast back to the weight dtype in SBUF)
    wrow = EPC * D * cfg.np_w.itemsize  # bytes per partition per panel
    wpad = 2 * wrow
    xt = nc.dram_tensor(
        "xt", [128, wpad + 2 * width], cfg.dt_x, kind="ExternalInput"
    ).ap()
    yt = nc.dram_tensor("yt", [128, 2 * width], cfg.dt_y, kind="ExternalOutput").ap()

    cast_engs = [{"d": nc.vector, "a": nc.scalar}[c] for c in cfg.cast_pat]
    st_engs = [
        {"g": nc.gpsimd, "a": nc.scalar, "s": nc.sync}[c] for c in cfg.st_pat
    ]

    with tile.TileContext(nc) as tc:
        with (
            tc.tile_pool(name="w", bufs=1) as wpool,
            tc.tile_pool(name="x", bufs=cfg.xbufs) as xpool,
            tc.tile_pool(name="y", bufs=cfg.ybufs) as ypool,
            tc.tile_pool(name="ps", bufs=cfg.psbufs, space="PSUM") as pspool,
        ):
            # stationary weights, shipped as raw bytes at the head of xt,
            # packed per segment: [w0_j | w1_j] blocks of wblk bytes.
            # Segment 0's block loads first on the scalar ring (in parallel
            # with the first token pieces on the sync ring) so the first
            # real matmul waits only ~0.8us of weight bytes.
            wblk = wpad // EPC  # bytes per segment (both K-halves)
            w01 = wpool.tile([128, wpad], cfg.dt_x, tag="w01")
            nc.scalar.dma_start(out=w01[:, 0:wblk], in_=xt[:, 0:wblk])
            nc.scalar.dma_start(out=w01[:, wblk:wpad], in_=xt[:, wblk:wpad])
            qb = wblk // 4  # bytes per [128,128] weight tile
            wap = [
                [
                    [
                        w01[:, j * wblk + h * 2 * qb + oc * qb :][:, 0:qb].bitcast(
                            cfg.dt_w
                        )
                        for oc in range(2)
                    ]
                    for h in range(2)
                ]
                for j in range(EPC)
            ]

            # dummy matmuls during the DMA-warmup window pull the PE out of
            # its low p-state before the first real chunk lands
            if cfg.warm_mms:
                wdum = wpool.tile([128, 16], cfg.dt_w, tag="wdum")
                xdum = wpool.tile([128, 512], cfg.dt_x, tag="xdum")
                nc.gpsimd.memset(wdum[:], 0)
                nc.gpsimd.memset(xdum[:], 0)
                # a few 16-col warm-ups gated only on the (tiny) wdum
                # memset get the PE churning ~0.4us before the 512-col
                # xdum memset completes
                wdum8 = wdum[:].bitcast(cfg.dt_x)
                for _ in range(3):
                    ps = pspool.tile([128, 512], mybir.dt.float32, tag="ps")
                    nc.tensor.matmul(
                        ps[0:16, 0:32], wdum[:], wdum8[:, 0:32], start=True, stop=True
                    )
                for _ in range(cfg.warm_mms):
                    ps = pspool.tile([128, 512], mybir.dt.float32, tag="ps")
                    nc.tensor.matmul(
                        ps[0:16, :], wdum[:], xdum[:], start=True, stop=True
                    )

            # x loads ride the sync ring, which does nothing else so its
            # DMA triggers never queue behind semaphore-waiting work (engine
            # streams are in-order). One early chunk rides the gpsimd ring
            # (its trigger precedes any store in that stream) so the supply
            # builds a cushion while the PE is still on warm-up matmuls.
            def _ld_eng(i):
                return nc.gpsimd if i == 3 else nc.sync

            castidx = 0
            chidx = 0
            ldidx = 0
            seg_base = 0
            nseg = len(segs)
            for j in range(nseg):
                seg = segs[j]
                chunks_j = _chunk_offsets(
                    seg, CHUNK, first_split=(j == 0), last_split=(j == nseg - 1)
                )
                for k, (coff, cw) in enumerate(chunks_j):
                    rev = len(chunks_j) - 1 - k if j == nseg - 1 else -1
                    bx = wpad + 2 * (seg_base + coff)  # xt block offset
                    b0 = 2 * (seg_base + coff)  # yt block offset
                    # tiles hold the piece contiguously ([2, cw] halves
                    # back-to-back) so every DMA is a single contiguous run
                    # per partition regardless of piece width
                    x01 = xpool.tile([128, 2 * CHUNK], cfg.dt_x, tag="x01")
                    ld_eng = _ld_eng(ldidx)
                    ldidx += 1
                    ld_eng.dma_start(
                        out=x01[:, 0 : 2 * cw], in_=xt[:, bx : bx + 2 * cw]
                    )
                    x0 = x01[:, 0:cw]
                    x1 = x01[:, cw : 2 * cw]
                    # the final chunk's pieces alternate their stores over
                    # the two HWDGE rings so the post-compute drain is short
                    # (the gpsimd SWDGE ring drains slowly at teardown)
                    is_tail = j == nseg - 1 and coff + 2 * CHUNK >= seg
                    ysb01 = ypool.tile([128, 2 * CHUNK], cfg.dt_y, tag="y01")

                    def _mm_pair(ps_dst, oc, soff, sw):
                        nc.tensor.matmul(
                            ps_dst,
                            wap[j][0][oc],
                            x0[:, soff : soff + sw],
                            start=True,
                            stop=False,
                        )
                        nc.tensor.matmul(
                            ps_dst,
                            wap[j][1][oc],
                            x1[:, soff : soff + sw],
                            start=False,
                            stop=True,
                        )

                    def _cast(dst, src, eng=None):
                        nonlocal castidx
                        if eng is None:
                            eng = cast_engs[castidx % len(cast_engs)]
                        castidx += 1
                        if eng is nc.scalar:
                            eng.copy(dst, src)
                        else:
                            eng.tensor_copy(dst, src)

                    if cfg.wide_cast and cw <= 512:
                        # small piece: both oc halves accumulate into the two
                        # banks of one wide PSUM tile; a single cast drains it
                        ps = pspool.tile([128, 1024], mybir.dt.float32, tag="ps")
                        for oc in range(2):
                            _mm_pair(ps[:, oc * 512 : oc * 512 + cw], oc, 0, cw)
                        if cw == 512:
                            _cast(ysb01[:, 0 : 2 * cw], ps[:])
                        else:
                            _cast(
                                ysb01[:, 0 : 2 * cw].rearrange(
                                    "p (b c) -> p b c", b=2
                                ),
                                ps[:].rearrange("p (b c) -> p b c", b=2)[
                                    :, :, 0:cw
                                ],
                            )
                    elif cfg.wide_cast:
                        # big chunk: adjacent 512-spans pair into one wide
                        # PSUM tile (contiguous columns), one cast per pair
                        for oc in range(2):
                            ysb = ysb01[:, oc * cw : (oc + 1) * cw]
                            spans = _splits(cw)
                            i = 0
                            while i < len(spans):
                                s0, w0 = spans[i]
                                ps = pspool.tile(
                                    [128, 1024], mybir.dt.float32, tag="ps"
                                )
                                _mm_pair(ps[:, 0:w0], oc, s0, w0)
                                tot = w0
                                i += 1
                                if w0 == 512 and i < len(spans):
                                    s1, w1 = spans[i]
                                    _mm_pair(ps[:, 512 : 512 + w1], oc, s1, w1)
                                    tot += w1
                                    i += 1
                                _cast(ysb[:, s0 : s0 + tot], ps[:, 0:tot])
                    else:
                        for oc in range(2):
                            ysb = ysb01[:, oc * cw : (oc + 1) * cw]
                            for soff, sw in _splits(cw):
                                ps = pspool.tile(
                                    [128, 512], mybir.dt.float32, tag="ps"
                                )
                                _mm_pair(ps[:, :sw], oc, soff, sw)
                                # the very last piece casts its halves on
                                # both engines in parallel so the final
                                # store isn't stuck behind one engine
                                feng = None
                                if rev == 0:
                                    feng = nc.scalar if oc == 0 else nc.vector
                                _cast(ysb[:, soff : soff + sw], ps[:, :sw], feng)
                    # single store per chunk (both oc halves); alternate
                    # rings so no single DMA queue limits the drain. Tail
                    # stores alternate the two HWDGE rings anchored so the
                    # FINAL piece rides the idle sync ring.
                    if is_tail:
                        st_eng = nc.scalar if rev % 2 == 1 else nc.sync
                    else:
                        st_eng = st_engs[chidx % len(st_engs)]
                    chidx += 1
                    st_eng.dma_start(
                        out=yt[:, b0 : b0 + 2 * cw], in_=ysb01[:, 0 : 2 * cw]
                    )
                seg_base += seg
    nc.compile()
    return nc


def _plan(counts):
    """Snake-assign experts to cores and derive shared segment sizes.

    Returns (groups, segs): groups[c] lists core c's experts largest-first;
    segs[j] = max over cores of the j-th largest count, rounded to SEG_GRAN.
    """
    order = np.argsort(-counts, kind="stable")
    groups = [[] for _ in range(NCORES)]
    for r, e in enumerate(order):
        band, pos = divmod(r, NCORES)
        c = pos if band % 2 == 0 else NCORES - 1 - pos
        groups[c].append(int(e))
    for g in groups:
        g.sort(key=lambda e: -counts[e])
    segs = tuple(
        int(-(-max(int(counts[g[j]]) for g in groups) // SEG_GRAN)) * SEG_GRAN
        for j in range(EPC)
    )
    return groups, segs


def kernel(inp, weight, fwd_expert_count, capacity):
    global last_exec_time_ns, last_results

    cfg = _Cfg()
    inp = np.asarray(inp)
    weight = np.asarray(weight)
    counts = np.asarray(fwd_expert_count).astype(np.int64)
    T, d_in = inp.shape
    E = weight.shape[0]
    assert d_in == D and E == NCORES * EPC
    assert int(counts.sum()) == T, "counts must cover all tokens"

    ends = np.cumsum(counts)
    starts = ends - counts
    groups, segs = _plan(counts)
    seg_off = [0]
    for s in segs:
        seg_off.append(seg_off[-1] + s)
    width = seg_off[-1]
    wrow = EPC * D * cfg.np_w.itemsize
    wpad = 2 * wrow

    # host-side scatter: transpose once, then contiguous row-slice copies
    xt_full = np.ascontiguousarray(inp.T)  # [D, T] float32
    if cfg.xscale != 1.0:
        xt_full = xt_full * np.float32(cfg.xscale)
    if cfg.np_x != np.float32:
        if cfg.np_x.itemsize == 1:
            xt_full = np.clip(xt_full, -15.5, 15.5)
        xt_full = xt_full.astype(cfg.np_x)

    in_maps = []
    for dcore in range(NCORES):
        # per-segment padded panel [D, width] in the old orientation
        xo = np.zeros((D, width), dtype=cfg.np_x)
        for j in range(EPC):
            e = groups[dcore][j]
            s, c = int(starts[e]), int(counts[e])
            xo[:, seg_off[j] : seg_off[j] + c] = xt_full[:, s : s + c]
        # chunk-major device layout [128, wpad + 2*width], w bytes first
        xd = np.empty((128, wpad + 2 * width), dtype=cfg.np_x)
        wl = weight[groups[dcore]]  # [EPC, out, in] in segment order
        wt = np.ascontiguousarray(wl.transpose(2, 0, 1).reshape(D, EPC * D))
        if cfg.xscale != 1.0:
            # x ships as xscale*x; fold 1/xscale into w so PSUM holds
            # unscaled y (e3m4 y-cast must stay within +-15.5)
            wt = wt * np.float32(1.0 / cfg.xscale)
        wb = wt.astype(cfg.np_w).view(np.uint8)  # [256, wrow]
        xdb = xd.view(np.uint8)
        wblk = wpad // EPC
        for j in range(EPC):
            eb = j * wrow // EPC
            xdb[:, j * wblk : j * wblk + wblk // 2] = wb[0:128, eb : eb + wblk // 2]
            xdb[:, j * wblk + wblk // 2 : (j + 1) * wblk] = wb[
                128:256, eb : eb + wblk // 2
            ]
        for j in range(EPC):
            for coff, cw in _chunk_offsets(
                segs[j], cfg.chunk, first_split=(j == 0), last_split=(j == EPC - 1)
            ):
                b0 = wpad + 2 * (seg_off[j] + coff)
                t0 = seg_off[j] + coff
                blk = xd[:, b0 : b0 + 2 * cw].reshape(128, 2, cw)
                blk[:, 0, :] = xo[0:128, t0 : t0 + cw]
                blk[:, 1, :] = xo[128:256, t0 : t0 + cw]
        in_maps.append({"xt": xd})

    key = cfg.key(segs)
    if key not in _prog_cache:
        _prog_cache[key] = _build_program(cfg, segs)
    nc = _prog_cache[key]

    trace = bool(int(os.environ.get("BASSMOE_TRACE", "0")))
    res = run_bass_kernel_spmd(nc, in_maps, list(range(NCORES)), trace=trace)
    last_exec_time_ns = res.exec_time_ns
    last_results = res

    # gather back to token order (y is unscaled: 1/xscale is folded into w)
    out_t = np.empty((D, T), dtype=np.float32)
    for dcore in range(NCORES):
        yd = np.asarray(res.results[dcore]["yt"]).astype(np.float32)
        for j in range(EPC):
            e = groups[dcore][j]
            s, c = int(starts[e]), int(counts[e])
            done = 0
            for coff, cw in _chunk_offsets(
                segs[j], cfg.chunk, first_split=(j == 0), last_split=(j == EPC - 1)
            ):
                if done >= c:
                    break
                take = min(cw, c - done)
                b0 = 2 * (seg_off[j] + coff)
                blk = yd[:, b0 : b0 + 2 * cw].reshape(128, 2, cw)
                out_t[0:128, s + done : s + done + take] = blk[:, 0, :take]
                out_t[128:256, s + done : s + done + take] = blk[:, 1, :take]
                done += take
            assert done >= c
    return np.ascontiguousarray(out_t.T)


# revision 54
# speedup vs baseline: 1.0218x; 1.0218x over previous
"""MoE grouped-GEMM (FMoELinear) on 8 trn2 NeuronCores.

Strategy (expert parallelism):
  - 32 experts, 8 cores -> 4 experts per core, chosen by a snake assignment
    over the count-sorted experts so per-core token totals balance.
  - Tokens arrive pre-sorted by expert. Each core's 4 experts map to 4
    fixed-size SEGMENTS (sizes shared by all cores so one SPMD program
    serves every core): segment j holds the core's j-th largest expert,
    and S_j = max over cores of the j-th largest count, rounded to 32.
    This cuts padding from a uniform per-expert cap to ~0.3%.
  - Device computes yt[o, t] = sum_i W[e][o, i] * x[t, i] per segment with
    the weight stationary in the PE array:
        lhsT = wt[i_chunk, j*256 + oc*128 : +128]   (128 x 128, stationary)
        rhs  = xt tile    [i_chunk, token span]     (128 x <=512, moving)
    accumulating the two i-chunks into PSUM, then casting PSUM->SBUF->HBM.
  - Host gathers the non-padded columns back into token order.

Precision/bandwidth plan: rel-err budget is 2e-2; stream x and y as fp8
E3M4 (~1.3% rms quant noise each; measured total 1.75e-2) to halve HBM
traffic and SBUF pressure. x is pre-scaled by XSCALE (folded back via
w/XSCALE) so fewer values land in the E3M4 subnormal range while PSUM y
stays unscaled (|y|max ~8.9 must fit E3M4's +-15.5 on the cast).

Engine layout: PE does 2 matmuls per token span (K=256 split in two
128-row chunks) at 1 cycle/column; the PSUM->SBUF casts alternate between
DVE and ACT. Only three DMA rings exist (sync/scalar HWDGE + gpsimd
SWDGE) at ~165-200 GB/s each, and engine instruction streams are
in-order, so x loads ride the sync ring (which does nothing else), the
weight panels and half the y stores ride the scalar ring, and the other
half of the stores ride the gpsimd ring. The first chunk is split into
512-token pieces and one early chunk is loaded via gpsimd so supply
outruns the PE during queue spin-up; warm-up matmuls cover that window
and pull the PE out of its low power-state. The final chunks store in
512-token pieces alternating scalar/sync rings, anchored so the very
last piece casts on both DVE and ACT in parallel and stores on the
then-idle sync ring (the gpsimd ring drains slowly at teardown).

DRAM layout: x and y are stored chunk-major as [128, sum_j 2*S_j] so every
full-chunk DMA is a single 2*CHUNK-byte contiguous run per partition
(bigger SDMA descriptors -> better per-queue DMA throughput). Block c at
token offset t0 holds tokens [t0, t0+cw) as [2, cw]: row 0 = in-features
0..127, row 1 = in-features 128..255 (for y: out-features).
"""

import os
import sys
import types

import ml_dtypes
import numpy as np

import concourse.bacc as bacc
import concourse.mybir as mybir
import concourse.tile as tile
from concourse.bass_utils import run_bass_kernel_spmd


def _ensure_axon_hooks_importable():
    """bass_utils imports antenv.axon_hooks when tracing is requested; some
    images lack that module. Provide a no-op fallback so a stray BASS_TRACE
    env var can't crash the kernel (tracing then degrades gracefully)."""
    try:
        import antenv  # noqa: F401
    except ImportError:
        return
    try:
        import antenv.axon_hooks  # noqa: F401
    except ImportError:
        mod = types.ModuleType("antenv.axon_hooks")
        holder = [None]
        mod.set_axon_ntff_profile_hook = lambda h: holder.__setitem__(0, h)
        mod.get_axon_ntff_profile_hook = lambda: holder[0]
        sys.modules["antenv.axon_hooks"] = mod
        import antenv as _antenv

        _antenv.axon_hooks = mod


_ensure_axon_hooks_importable()

NCORES = 8
D = 256  # in/out feature dim
EPC = 4  # experts per core
SEG_GRAN = 32  # segment size granularity (tokens)

# observability for test harness
last_exec_time_ns = None
last_results = None

_prog_cache = {}


def _dt1(name):
    if name == "f32":
        return mybir.dt.float32, np.dtype(np.float32)
    if name == "f32r":
        return mybir.dt.float32r, np.dtype(np.float32)
    if name == "f16":
        return mybir.dt.float16, np.dtype(np.float16)
    if name == "bf16":
        return mybir.dt.bfloat16, np.dtype(ml_dtypes.bfloat16)
    if name == "f8e3":
        return mybir.dt.float8e3, np.dtype(ml_dtypes.float8_e3m4)
    if name == "f8e4":
        return mybir.dt.float8e4, np.dtype(ml_dtypes.float8_e4m3)
    if name == "f8e5":
        return mybir.dt.float8e5, np.dtype(ml_dtypes.float8_e5m2)
    raise ValueError(name)


class _Cfg:
    def __init__(self):
        # "xdt" or "xdt+wdt": moving (x) and stationary (w) matmul dtypes
        self.mm_dt = os.environ.get("BASSMOE_MM_DT", "f8e3+f16")
        self.y_dt = os.environ.get("BASSMOE_Y_DT", "f8e3")
        self.xscale = float(os.environ.get("BASSMOE_XSCALE", "2"))
        self.chunk = int(os.environ.get("BASSMOE_CHUNK", "2048"))
        # cast-engine pattern, cycled per PSUM-bank cast: d=DVE, a=ACT
        self.cast_pat = os.environ.get("BASSMOE_CAST_PAT", "da")
        # engines issuing y stores, cycled per chunk: g=gpsimd, a=ACT, s=SP
        self.st_pat = os.environ.get("BASSMOE_ST_PAT", "ga")
        self.xbufs = int(os.environ.get("BASSMOE_XBUFS", "8"))
        self.ybufs = int(os.environ.get("BASSMOE_YBUFS", "6"))
        self.psbufs = int(os.environ.get("BASSMOE_PSBUFS", "8"))
        self.wide_cast = bool(int(os.environ.get("BASSMOE_WIDE_CAST", "0")))
        self.warm_mms = int(os.environ.get("BASSMOE_WARM_MMS", "6"))
        self.warm_inter = int(os.environ.get("BASSMOE_WARM_INTER", "0"))
        parts = self.mm_dt.split("+")
        self.dt_x, self.np_x = _dt1(parts[0])
        self.dt_w, self.np_w = _dt1(parts[-1])
        self.dt_y, self.np_y = _dt1(self.y_dt)

    def key(self, segs):
        return (
            segs,
            self.mm_dt,
            self.y_dt,
            self.chunk,
            self.cast_pat,
            self.st_pat,
            self.xbufs,
            self.ybufs,
            self.psbufs,
            self.warm_mms,
            self.warm_inter,
            self.wide_cast,
        )


def _chunk_offsets(
    seg: int, chunk: int, first_split: bool = False, last_split: bool = False
):
    """(offset, width) chunks covering [0, seg), width <= chunk.

    first_split breaks the leading chunk into small pieces so the very
    first matmul can start as soon as a short prefix of x has landed;
    last_split tapers the trailing chunks the same way so the final store
    transfers (which serialize after the last casts) are short."""
    out = []
    off = 0
    while off < seg:
        w = min(chunk, seg - off)
        if first_split and off == 0:
            # two 1024-token pieces: small enough that the first matmul
            # starts early, few enough that the serial ~0.7us DMA-trigger
            # instructions don't delay the following chunk loads
            s = 0
            while s < w:
                out.append((off + s, min(1024, w - s)))
                s += 1024
        elif last_split and off + 2 * chunk >= seg:
            s = 0
            while s < w:
                r = w - s
                if off + w == seg and r <= 512:
                    # the very last piece: split small so the final
                    # cast->store chain after the last matmul is short
                    if r > 256:
                        out.append((off + s, r - 192))
                        s += r - 192
                        r = 192
                    out.append((off + s, r))
                    s += r
                else:
                    out.append((off + s, min(512, r)))
                    s += 512
        else:
            out.append((off, w))
        off += w
    return out


def _splits(width: int):
    """(offset, width) matmul spans <= 512 covering [0, width)."""
    out = []
    off = 0
    while off < width:
        w = min(512, width - off)
        out.append((off, w))
        off += w
    return out


def _build_program(cfg: _Cfg, segs: tuple):
    """Build the SPMD Bass program for per-segment token sizes `segs`."""
    width = sum(segs)
    CHUNK = cfg.chunk

    nc = bacc.Bacc(
        "TRN2",
        target_bir_lowering=False,
        debug=False,
        enable_asserts=False,
        num_devices=NCORES,
    )
    # chunk-major layout: [128, wpad + 2*width]; the first wpad columns hold
    # the raw bytes of the four weight panels so they ride ahead of the
    # token stream (bitcast back to the weight dtype in SBUF)
    wrow = EPC * D * cfg.np_w.itemsize  # bytes per partition per panel
    wpad = 2 * wrow
    xt = nc.dram_tensor(
        "xt", [128, wpad + 2 * width], cfg.dt_x, kind="ExternalInput"
    ).ap()
    yt = nc.dram_tensor("yt", [128, 2 * width], cfg.dt_y, kind="ExternalOutput").ap()

    cast_engs = [{"d": nc.vector, "a": nc.scalar}[c] for c in cfg.cast_pat]
    st_engs = [
        {"g": nc.gpsimd, "a": nc.scalar, "s": nc.sync}[c] for c in cfg.st_pat
    ]

    with tile.TileContext(nc) as tc:
        with (
            tc.tile_pool(name="w", bufs=1) as wpool,
            tc.tile_pool(name="x", bufs=cfg.xbufs) as xpool,
            tc.tile_pool(name="y", bufs=cfg.ybufs) as ypool,
            tc.tile_pool(name="ps", bufs=cfg.psbufs, space="PSUM") as pspool,
        ):
            # stationary weights, shipped as raw bytes at the head of xt,
            # packed per segment: [w0_j | w1_j] blocks of wblk bytes.
            # Segment 0's block loads first on the scalar ring (in parallel
            # with the first token pieces on the sync ring) so the first
            # real matmul waits only ~0.8us of weight bytes.
            wblk = wpad // EPC  # bytes per segment (both K-halves)
            w01 = wpool.tile([128, wpad], cfg.dt_x, tag="w01")
            # segment 0's panel loads first; the other three panels are
            # deferred into the chunk loop (they aren't needed until
            # segment 1, ~40us in) so the scalar ring can deliver an
            # early x chunk instead of the slow-starting gpsimd SWDGE
            nc.scalar.dma_start(out=w01[:, 0:wblk], in_=xt[:, 0:wblk])
            qb = wblk // 4  # bytes per [128,128] weight tile
            wap = [
                [
                    [
                        w01[:, j * wblk + h * 2 * qb + oc * qb :][:, 0:qb].bitcast(
                            cfg.dt_w
                        )
                        for oc in range(2)
                    ]
                    for h in range(2)
                ]
                for j in range(EPC)
            ]

            # dummy matmuls during the DMA-warmup window pull the PE out of
            # its low p-state before the first real chunk lands
            if cfg.warm_mms:
                wdum = wpool.tile([128, 16], cfg.dt_w, tag="wdum")
                xdum = wpool.tile([128, 512], cfg.dt_x, tag="xdum")
                nc.gpsimd.memset(wdum[:], 0)
                nc.gpsimd.memset(xdum[:], 0)
                for _ in range(cfg.warm_mms):
                    ps = pspool.tile([128, 512], mybir.dt.float32, tag="ps")
                    nc.tensor.matmul(
                        ps[0:16, :], wdum[:], xdum[:], start=True, stop=True
                    )

            # x loads ride the sync ring, which does nothing else so its
            # DMA triggers never queue behind semaphore-waiting work (engine
            # streams are in-order). One early chunk rides the gpsimd ring
            # (its trigger precedes any store in that stream) so the supply
            # builds a cushion while the PE is still on warm-up matmuls.
            def _ld_eng(i):
                return nc.scalar if i == 2 else nc.sync

            castidx = 0
            chidx = 0
            ldidx = 0
            seg_base = 0
            nseg = len(segs)
            for j in range(nseg):
                seg = segs[j]
                chunks_j = _chunk_offsets(
                    seg, CHUNK, first_split=(j == 0), last_split=(j == nseg - 1)
                )
                for k, (coff, cw) in enumerate(chunks_j):
                    rev = len(chunks_j) - 1 - k if j == nseg - 1 else -1
                    if ldidx == 5:
                        # deferred panels 1-3 ride scalar behind c1
                        nc.scalar.dma_start(
                            out=w01[:, wblk:wpad], in_=xt[:, wblk:wpad]
                        )
                    bx = wpad + 2 * (seg_base + coff)  # xt block offset
                    b0 = 2 * (seg_base + coff)  # yt block offset
                    # tiles hold the piece contiguously ([2, cw] halves
                    # back-to-back) so every DMA is a single contiguous run
                    # per partition regardless of piece width
                    x01 = xpool.tile([128, 2 * CHUNK], cfg.dt_x, tag="x01")
                    ld_eng = _ld_eng(ldidx)
                    ldidx += 1
                    ld_eng.dma_start(
                        out=x01[:, 0 : 2 * cw], in_=xt[:, bx : bx + 2 * cw]
                    )
                    x0 = x01[:, 0:cw]
                    x1 = x01[:, cw : 2 * cw]
                    # the final chunk's pieces alternate their stores over
                    # the two HWDGE rings so the post-compute drain is short
                    # (the gpsimd SWDGE ring drains slowly at teardown)
                    is_tail = j == nseg - 1 and coff + 2 * CHUNK >= seg
                    ysb01 = ypool.tile([128, 2 * CHUNK], cfg.dt_y, tag="y01")

                    def _mm_pair(ps_dst, oc, soff, sw):
                        nc.tensor.matmul(
                            ps_dst,
                            wap[j][0][oc],
                            x0[:, soff : soff + sw],
                            start=True,
                            stop=False,
                        )
                        nc.tensor.matmul(
                            ps_dst,
                            wap[j][1][oc],
                            x1[:, soff : soff + sw],
                            start=False,
                            stop=True,
                        )

                    def _cast(dst, src, eng=None):
                        nonlocal castidx
                        if eng is None:
                            eng = cast_engs[castidx % len(cast_engs)]
                        castidx += 1
                        if eng is nc.scalar:
                            eng.copy(dst, src)
                        else:
                            eng.tensor_copy(dst, src)

                    if cfg.wide_cast and cw <= 512:
                        # small piece: both oc halves accumulate into the two
                        # banks of one wide PSUM tile; a single cast drains it
                        ps = pspool.tile([128, 1024], mybir.dt.float32, tag="ps")
                        for oc in range(2):
                            _mm_pair(ps[:, oc * 512 : oc * 512 + cw], oc, 0, cw)
                        if cw == 512:
                            _cast(ysb01[:, 0 : 2 * cw], ps[:])
                        else:
                            _cast(
                                ysb01[:, 0 : 2 * cw].rearrange(
                                    "p (b c) -> p b c", b=2
                                ),
                                ps[:].rearrange("p (b c) -> p b c", b=2)[
                                    :, :, 0:cw
                                ],
                            )
                    elif cfg.wide_cast:
                        # big chunk: adjacent 512-spans pair into one wide
                        # PSUM tile (contiguous columns), one cast per pair
                        for oc in range(2):
                            ysb = ysb01[:, oc * cw : (oc + 1) * cw]
                            spans = _splits(cw)
                            i = 0
                            while i < len(spans):
                                s0, w0 = spans[i]
                                ps = pspool.tile(
                                    [128, 1024], mybir.dt.float32, tag="ps"
                                )
                                _mm_pair(ps[:, 0:w0], oc, s0, w0)
                                tot = w0
                                i += 1
                                if w0 == 512 and i < len(spans):
                                    s1, w1 = spans[i]
                                    _mm_pair(ps[:, 512 : 512 + w1], oc, s1, w1)
                                    tot += w1
                                    i += 1
                                _cast(ysb[:, s0 : s0 + tot], ps[:, 0:tot])
                    else:
                        for oc in range(2):
                            ysb = ysb01[:, oc * cw : (oc + 1) * cw]
                            for soff, sw in _splits(cw):
                                ps = pspool.tile(
                                    [128, 512], mybir.dt.float32, tag="ps"
                                )
                                _mm_pair(ps[:, :sw], oc, soff, sw)
                                # the very last piece casts its halves on
                                # both engines in parallel so the final
                                # store isn't stuck behind one engine
                                feng = None
                                if rev == 0:
                                    feng = nc.scalar if oc == 0 else nc.vector
                                _cast(ysb[:, soff : soff + sw], ps[:, :sw], feng)
                    # single store per chunk (both oc halves); alternate
                    # rings so no single DMA queue limits the drain. Tail
                    # stores alternate the two HWDGE rings anchored so the
                    # FINAL piece rides the idle sync ring.
                    if is_tail:
                        st_eng = nc.scalar if rev % 2 == 1 else nc.sync
                    else:
                        st_eng = st_engs[chidx % len(st_engs)]
                    chidx += 1
                    st_eng.dma_start(
                        out=yt[:, b0 : b0 + 2 * cw], in_=ysb01[:, 0 : 2 * cw]
                    )
                seg_base += seg
    nc.compile()
    return nc


def _plan(counts):
    """Snake-assign experts to cores and derive shared segment sizes.

    Returns (groups, segs): groups[c] lists core c's experts largest-first;
    segs[j] = max over cores of the j-th largest count, rounded to SEG_GRAN.
    """
    order = np.argsort(-counts, kind="stable")
    groups = [[] for _ in range(NCORES)]
    for r, e in enumerate(order):
        band, pos = divmod(r, NCORES)
        c = pos if band % 2 == 0 else NCORES - 1 - pos
        groups[c].append(int(e))
    for g in groups:
        g.sort(key=lambda e: -counts[e])
    segs = tuple(
        int(-(-max(int(counts[g[j]]) for g in groups) // SEG_GRAN)) * SEG_GRAN
        for j in range(EPC)
    )
    return groups, segs


def kernel(inp, weight, fwd_expert_count, capacity):
    global last_exec_time_ns, last_results

    cfg = _Cfg()
    inp = np.asarray(inp)
    weight = np.asarray(weight)
    counts = np.asarray(fwd_expert_count).astype(np.int64)
    T, d_in = inp.shape
    E = weight.shape[0]
    assert d_in == D and E == NCORES * EPC
    assert int(counts.sum()) == T, "counts must cover all tokens"

    ends = np.cumsum(counts)
    starts = ends - counts
    groups, segs = _plan(counts)
    seg_off = [0]
    for s in segs:
        seg_off.append(seg_off[-1] + s)
    width = seg_off[-1]
    wrow = EPC * D * cfg.np_w.itemsize
    wpad = 2 * wrow

    # host-side scatter: transpose once, then contiguous row-slice copies
    xt_full = np.ascontiguousarray(inp.T)  # [D, T] float32
    if cfg.xscale != 1.0:
        xt_full = xt_full * np.float32(cfg.xscale)
    if cfg.np_x != np.float32:
        if cfg.np_x.itemsize == 1:
            xt_full = np.clip(xt_full, -15.5, 15.5)
        xt_full = xt_full.astype(cfg.np_x)

    in_maps = []
    for dcore in range(NCORES):
        # per-segment padded panel [D, width] in the old orientation
        xo = np.zeros((D, width), dtype=cfg.np_x)
        for j in range(EPC):
            e = groups[dcore][j]
            s, c = int(starts[e]), int(counts[e])
            xo[:, seg_off[j] : seg_off[j] + c] = xt_full[:, s : s + c]
        # chunk-major device layout [128, wpad + 2*width], w bytes first
        xd = np.empty((128, wpad + 2 * width), dtype=cfg.np_x)
        wl = weight[groups[dcore]]  # [EPC, out, in] in segment order
        wt = np.ascontiguousarray(wl.transpose(2, 0, 1).reshape(D, EPC * D))
        if cfg.xscale != 1.0:
            # x ships as xscale*x; fold 1/xscale into w so PSUM holds
            # unscaled y (e3m4 y-cast must stay within +-15.5)
            wt = wt * np.float32(1.0 / cfg.xscale)
        wb = wt.astype(cfg.np_w).view(np.uint8)  # [256, wrow]
        xdb = xd.view(np.uint8)
        wblk = wpad // EPC
        for j in range(EPC):
            eb = j * wrow // EPC
            xdb[:, j * wblk : j * wblk + wblk // 2] = wb[0:128, eb : eb + wblk // 2]
            xdb[:, j * wblk + wblk // 2 : (j + 1) * wblk] = wb[
                128:256, eb : eb + wblk // 2
            ]
        for j in range(EPC):
            for coff, cw in _chunk_offsets(
                segs[j], cfg.chunk, first_split=(j == 0), last_split=(j == EPC - 1)
            ):
                b0 = wpad + 2 * (seg_off[j] + coff)
                t0 = seg_off[j] + coff
                blk = xd[:, b0 : b0 + 2 * cw].reshape(128, 2, cw)
                blk[:, 0, :] = xo[0:128, t0 : t0 + cw]
                blk[:, 1, :] = xo[128:256, t0 : t0 + cw]
        in_maps.append({"xt": xd})

    key = cfg.key(segs)
    if key not in _prog_cache:
        _prog_cache[key] = _build_program(cfg, segs)
    nc = _prog_cache[key]

    trace = bool(int(os.environ.get("BASSMOE_TRACE", "0")))
    res = run_bass_kernel_spmd(nc, in_maps, list(range(NCORES)), trace=trace)
    last_exec_time_ns = res.exec_time_ns
    last_results = res

    # gather back to token order (y is unscaled: 1/xscale is folded into w)
    out_t = np.empty((D, T), dtype=np.float32)
    for dcore in range(NCORES):
        yd = np.asarray(res.results[dcore]["yt"]).astype(np.float32)
        for j in range(EPC):
            e = groups[dcore][j]
            s, c = int(starts[e]), int(counts[e])
            done = 0
            for coff, cw in _chunk_offsets(
                segs[j], cfg.chunk, first_split=(j == 0), last_split=(j == EPC - 1)
            ):
                if done >= c:
                    break
                take = min(cw, c - done)
                b0 = 2 * (seg_off[j] + coff)
                blk = yd[:, b0 : b0 + 2 * cw].reshape(128, 2, cw)
                out_t[0:128, s + done : s + done + take] = blk[:, 0, :take]
                out_t[128:256, s + done : s + done + take] = blk[:, 1, :take]
                done += take
            assert done >= c
    return np.ascontiguousarray(out_t.T)


# revision 57
# speedup vs baseline: 1.0448x; 1.0225x over previous
"""MoE grouped-GEMM (FMoELinear) on 8 trn2 NeuronCores.

Strategy (expert parallelism):
  - 32 experts, 8 cores -> 4 experts per core, chosen by a snake assignment
    over the count-sorted experts so per-core token totals balance.
  - Tokens arrive pre-sorted by expert. Each core's 4 experts map to 4
    fixed-size SEGMENTS (sizes shared by all cores so one SPMD program
    serves every core): segment j holds the core's j-th largest expert,
    and S_j = max over cores of the j-th largest count, rounded to 32.
    This cuts padding from a uniform per-expert cap to ~0.3%.
  - Device computes yt[o, t] = sum_i W[e][o, i] * x[t, i] per segment with
    the weight stationary in the PE array:
        lhsT = wt[i_chunk, j*256 + oc*128 : +128]   (128 x 128, stationary)
        rhs  = xt tile    [i_chunk, token span]     (128 x <=512, moving)
    accumulating the two i-chunks into PSUM, then casting PSUM->SBUF->HBM.
  - Host gathers the non-padded columns back into token order.

Precision/bandwidth plan: rel-err budget is 2e-2; stream x and y as fp8
E3M4 (~1.3% rms quant noise each; measured total 1.75e-2) to halve HBM
traffic and SBUF pressure. x is pre-scaled by XSCALE (folded back via
w/XSCALE) so fewer values land in the E3M4 subnormal range while PSUM y
stays unscaled (|y|max ~8.9 must fit E3M4's +-15.5 on the cast).

Engine layout: PE does 2 matmuls per token span (K=256 split in two
128-row chunks) at 1 cycle/column; the PSUM->SBUF casts alternate between
DVE and ACT. Only three DMA rings exist (sync/scalar HWDGE + gpsimd
SWDGE) at ~165-200 GB/s each, and engine instruction streams are
in-order, so x loads ride the sync ring (which does nothing else), the
weight panels and half the y stores ride the scalar ring, and the other
half of the stores ride the gpsimd ring. The first chunk is split into
512-token pieces and one early chunk is loaded via gpsimd so supply
outruns the PE during queue spin-up; warm-up matmuls cover that window
and pull the PE out of its low power-state. The final chunks store in
512-token pieces alternating scalar/sync rings, anchored so the very
last piece casts on both DVE and ACT in parallel and stores on the
then-idle sync ring (the gpsimd ring drains slowly at teardown).

DRAM layout: x and y are stored chunk-major as [128, sum_j 2*S_j] so every
full-chunk DMA is a single 2*CHUNK-byte contiguous run per partition
(bigger SDMA descriptors -> better per-queue DMA throughput). Block c at
token offset t0 holds tokens [t0, t0+cw) as [2, cw]: row 0 = in-features
0..127, row 1 = in-features 128..255 (for y: out-features).
"""

import os
import sys
import types

import ml_dtypes
import numpy as np

import concourse.bacc as bacc
import concourse.mybir as mybir
import concourse.tile as tile
from concourse.bass_utils import run_bass_kernel_spmd


def _ensure_axon_hooks_importable():
    """bass_utils imports antenv.axon_hooks when tracing is requested; some
    images lack that module. Provide a no-op fallback so a stray BASS_TRACE
    env var can't crash the kernel (tracing then degrades gracefully)."""
    try:
        import antenv  # noqa: F401
    except ImportError:
        return
    try:
        import antenv.axon_hooks  # noqa: F401
    except ImportError:
        mod = types.ModuleType("antenv.axon_hooks")
        holder = [None]
        mod.set_axon_ntff_profile_hook = lambda h: holder.__setitem__(0, h)
        mod.get_axon_ntff_profile_hook = lambda: holder[0]
        sys.modules["antenv.axon_hooks"] = mod
        import antenv as _antenv

        _antenv.axon_hooks = mod


_ensure_axon_hooks_importable()

NCORES = 8
D = 256  # in/out feature dim
EPC = 4  # experts per core
SEG_GRAN = 16  # segment size granularity (tokens)

# observability for test harness
last_exec_time_ns = None
last_results = None

_prog_cache = {}


def _dt1(name):
    if name == "f32":
        return mybir.dt.float32, np.dtype(np.float32)
    if name == "f32r":
        return mybir.dt.float32r, np.dtype(np.float32)
    if name == "f16":
        return mybir.dt.float16, np.dtype(np.float16)
    if name == "bf16":
        return mybir.dt.bfloat16, np.dtype(ml_dtypes.bfloat16)
    if name == "f8e3":
        return mybir.dt.float8e3, np.dtype(ml_dtypes.float8_e3m4)
    if name == "f8e4":
        return mybir.dt.float8e4, np.dtype(ml_dtypes.float8_e4m3)
    if name == "f8e5":
        return mybir.dt.float8e5, np.dtype(ml_dtypes.float8_e5m2)
    raise ValueError(name)


class _Cfg:
    def __init__(self):
        # "xdt" or "xdt+wdt": moving (x) and stationary (w) matmul dtypes
        self.mm_dt = os.environ.get("BASSMOE_MM_DT", "f8e3+f16")
        self.y_dt = os.environ.get("BASSMOE_Y_DT", "f8e3")
        self.xscale = float(os.environ.get("BASSMOE_XSCALE", "2"))
        self.chunk = int(os.environ.get("BASSMOE_CHUNK", "2048"))
        # cast-engine pattern, cycled per PSUM-bank cast: d=DVE, a=ACT
        self.cast_pat = os.environ.get("BASSMOE_CAST_PAT", "da")
        # engines issuing y stores, cycled per chunk: g=gpsimd, a=ACT, s=SP
        self.st_pat = os.environ.get("BASSMOE_ST_PAT", "ga")
        self.xbufs = int(os.environ.get("BASSMOE_XBUFS", "8"))
        self.ybufs = int(os.environ.get("BASSMOE_YBUFS", "6"))
        self.psbufs = int(os.environ.get("BASSMOE_PSBUFS", "8"))
        self.wide_cast = bool(int(os.environ.get("BASSMOE_WIDE_CAST", "0")))
        self.warm_mms = int(os.environ.get("BASSMOE_WARM_MMS", "7"))
        self.warm_inter = int(os.environ.get("BASSMOE_WARM_INTER", "0"))
        parts = self.mm_dt.split("+")
        self.dt_x, self.np_x = _dt1(parts[0])
        self.dt_w, self.np_w = _dt1(parts[-1])
        self.dt_y, self.np_y = _dt1(self.y_dt)

    def key(self, segs):
        return (
            segs,
            self.mm_dt,
            self.y_dt,
            self.chunk,
            self.cast_pat,
            self.st_pat,
            self.xbufs,
            self.ybufs,
            self.psbufs,
            self.warm_mms,
            self.warm_inter,
            self.wide_cast,
        )


def _chunk_offsets(
    seg: int, chunk: int, first_split: bool = False, last_split: bool = False
):
    """(offset, width) chunks covering [0, seg), width <= chunk.

    first_split breaks the leading chunk into small pieces so the very
    first matmul can start as soon as a short prefix of x has landed;
    last_split tapers the trailing chunks the same way so the final store
    transfers (which serialize after the last casts) are short."""
    out = []
    off = 0
    while off < seg:
        w = min(chunk, seg - off)
        if first_split and off == 0:
            # two 1024-token pieces: small enough that the first matmul
            # starts early, few enough that the serial ~0.7us DMA-trigger
            # instructions don't delay the following chunk loads
            s = 0
            while s < w:
                out.append((off + s, min(1024, w - s)))
                s += 1024
        elif last_split and off + 2 * chunk >= seg:
            s = 0
            while s < w:
                r = w - s
                if off + w == seg and r <= 512:
                    # the very last piece: split small so the final
                    # cast->store chain after the last matmul is short
                    if r > 256:
                        out.append((off + s, r - 192))
                        s += r - 192
                        r = 192
                    out.append((off + s, r))
                    s += r
                else:
                    out.append((off + s, min(512, r)))
                    s += 512
        else:
            out.append((off, w))
        off += w
    return out


def _splits(width: int):
    """(offset, width) matmul spans <= 512 covering [0, width)."""
    out = []
    off = 0
    while off < width:
        w = min(512, width - off)
        out.append((off, w))
        off += w
    return out


def _build_program(cfg: _Cfg, segs: tuple):
    """Build the SPMD Bass program for per-segment token sizes `segs`."""
    width = sum(segs)
    CHUNK = cfg.chunk

    nc = bacc.Bacc(
        "TRN2",
        target_bir_lowering=False,
        debug=False,
        enable_asserts=False,
        num_devices=NCORES,
    )
    # chunk-major layout: [128, wpad + 2*width]; the first wpad columns hold
    # the raw bytes of the four weight panels so they ride ahead of the
    # token stream (bitcast back to the weight dtype in SBUF)
    wrow = EPC * D * cfg.np_w.itemsize  # bytes per partition per panel
    wpad = 2 * wrow
    xt = nc.dram_tensor(
        "xt", [128, wpad + 2 * width], cfg.dt_x, kind="ExternalInput"
    ).ap()
    yt = nc.dram_tensor("yt", [128, 2 * width], cfg.dt_y, kind="ExternalOutput").ap()

    cast_engs = [{"d": nc.vector, "a": nc.scalar}[c] for c in cfg.cast_pat]
    st_engs = [
        {"g": nc.gpsimd, "a": nc.scalar, "s": nc.sync}[c] for c in cfg.st_pat
    ]

    with tile.TileContext(nc) as tc:
        with (
            tc.tile_pool(name="w", bufs=1) as wpool,
            tc.tile_pool(name="x", bufs=cfg.xbufs) as xpool,
            tc.tile_pool(name="y", bufs=cfg.ybufs) as ypool,
            tc.tile_pool(name="ps", bufs=cfg.psbufs, space="PSUM") as pspool,
        ):
            # stationary weights, shipped as raw bytes at the head of xt,
            # packed per segment: [w0_j | w1_j] blocks of wblk bytes.
            # Segment 0's block loads first on the scalar ring (in parallel
            # with the first token pieces on the sync ring) so the first
            # real matmul waits only ~0.8us of weight bytes.
            wblk = wpad // EPC  # bytes per segment (both K-halves)
            w01 = wpool.tile([128, wpad], cfg.dt_x, tag="w01")
            nc.scalar.dma_start(out=w01[:, 0:wblk], in_=xt[:, 0:wblk])
            nc.scalar.dma_start(out=w01[:, wblk:wpad], in_=xt[:, wblk:wpad])
            qb = wblk // 4  # bytes per [128,128] weight tile
            wap = [
                [
                    [
                        w01[:, j * wblk + h * 2 * qb + oc * qb :][:, 0:qb].bitcast(
                            cfg.dt_w
                        )
                        for oc in range(2)
                    ]
                    for h in range(2)
                ]
                for j in range(EPC)
            ]

            # dummy matmuls during the DMA-warmup window pull the PE out of
            # its low p-state before the first real chunk lands
            if cfg.warm_mms:
                wdum = wpool.tile([128, 16], cfg.dt_w, tag="wdum")
                xdum = wpool.tile([128, 512], cfg.dt_x, tag="xdum")
                nc.gpsimd.memset(wdum[:], 0)
                nc.gpsimd.memset(xdum[:], 0)
                for _ in range(cfg.warm_mms):
                    ps = pspool.tile([128, 512], mybir.dt.float32, tag="ps")
                    nc.tensor.matmul(
                        ps[0:16, :], wdum[:], xdum[:], start=True, stop=True
                    )

            # x loads ride the sync ring, which does nothing else so its
            # DMA triggers never queue behind semaphore-waiting work (engine
            # streams are in-order). One early chunk rides the gpsimd ring
            # (its trigger precedes any store in that stream) so the supply
            # builds a cushion while the PE is still on warm-up matmuls.
            def _ld_eng(i):
                return nc.gpsimd if i == 2 else nc.sync

            castidx = 0
            chidx = 0
            ldidx = 0
            seg_base = 0
            nseg = len(segs)
            for j in range(nseg):
                seg = segs[j]
                chunks_j = _chunk_offsets(
                    seg, CHUNK, first_split=(j == 0), last_split=(j == nseg - 1)
                )
                for k, (coff, cw) in enumerate(chunks_j):
                    rev = len(chunks_j) - 1 - k if j == nseg - 1 else -1
                    bx = wpad + 2 * (seg_base + coff)  # xt block offset
                    b0 = 2 * (seg_base + coff)  # yt block offset
                    # tiles hold the piece contiguously ([2, cw] halves
                    # back-to-back) so every DMA is a single contiguous run
                    # per partition regardless of piece width
                    x01 = xpool.tile([128, 2 * CHUNK], cfg.dt_x, tag="x01")
                    ld_eng = _ld_eng(ldidx)
                    ldidx += 1
                    ld_eng.dma_start(
                        out=x01[:, 0 : 2 * cw], in_=xt[:, bx : bx + 2 * cw]
                    )
                    x0 = x01[:, 0:cw]
                    x1 = x01[:, cw : 2 * cw]
                    # the final chunk's pieces alternate their stores over
                    # the two HWDGE rings so the post-compute drain is short
                    # (the gpsimd SWDGE ring drains slowly at teardown)
                    is_tail = j == nseg - 1 and coff + 2 * CHUNK >= seg
                    ysb01 = ypool.tile([128, 2 * CHUNK], cfg.dt_y, tag="y01")

                    def _mm_pair(ps_dst, oc, soff, sw):
                        nc.tensor.matmul(
                            ps_dst,
                            wap[j][0][oc],
                            x0[:, soff : soff + sw],
                            start=True,
                            stop=False,
                        )
                        nc.tensor.matmul(
                            ps_dst,
                            wap[j][1][oc],
                            x1[:, soff : soff + sw],
                            start=False,
                            stop=True,
                        )

                    def _cast(dst, src, eng=None):
                        nonlocal castidx
                        if eng is None:
                            eng = cast_engs[castidx % len(cast_engs)]
                        castidx += 1
                        if eng is nc.scalar:
                            eng.copy(dst, src)
                        else:
                            eng.tensor_copy(dst, src)

                    if cfg.wide_cast and cw <= 512:
                        # small piece: both oc halves accumulate into the two
                        # banks of one wide PSUM tile; a single cast drains it
                        ps = pspool.tile([128, 1024], mybir.dt.float32, tag="ps")
                        for oc in range(2):
                            _mm_pair(ps[:, oc * 512 : oc * 512 + cw], oc, 0, cw)
                        if cw == 512:
                            _cast(ysb01[:, 0 : 2 * cw], ps[:])
                        else:
                            _cast(
                                ysb01[:, 0 : 2 * cw].rearrange(
                                    "p (b c) -> p b c", b=2
                                ),
                                ps[:].rearrange("p (b c) -> p b c", b=2)[
                                    :, :, 0:cw
                                ],
                            )
                    elif cfg.wide_cast:
                        # big chunk: adjacent 512-spans pair into one wide
                        # PSUM tile (contiguous columns), one cast per pair
                        for oc in range(2):
                            ysb = ysb01[:, oc * cw : (oc + 1) * cw]
                            spans = _splits(cw)
                            i = 0
                            while i < len(spans):
                                s0, w0 = spans[i]
                                ps = pspool.tile(
                                    [128, 1024], mybir.dt.float32, tag="ps"
                                )
                                _mm_pair(ps[:, 0:w0], oc, s0, w0)
                                tot = w0
                                i += 1
                                if w0 == 512 and i < len(spans):
                                    s1, w1 = spans[i]
                                    _mm_pair(ps[:, 512 : 512 + w1], oc, s1, w1)
                                    tot += w1
                                    i += 1
                                _cast(ysb[:, s0 : s0 + tot], ps[:, 0:tot])
                    else:
                        for oc in range(2):
                            ysb = ysb01[:, oc * cw : (oc + 1) * cw]
                            for soff, sw in _splits(cw):
                                ps = pspool.tile(
                                    [128, 512], mybir.dt.float32, tag="ps"
                                )
                                _mm_pair(ps[:, :sw], oc, soff, sw)
                                # the very last piece casts its halves on
                                # both engines in parallel so the final
                                # store isn't stuck behind one engine
                                feng = None
                                if rev == 0:
                                    feng = nc.scalar if oc == 0 else nc.vector
                                _cast(ysb[:, soff : soff + sw], ps[:, :sw], feng)
                    # single store per chunk (both oc halves); alternate
                    # rings so no single DMA queue limits the drain. Tail
                    # stores alternate the two HWDGE rings anchored so the
                    # FINAL piece rides the idle sync ring.
                    if is_tail:
                        st_eng = nc.scalar if rev % 2 == 1 else nc.sync
                    else:
                        st_eng = st_engs[chidx % len(st_engs)]
                    chidx += 1
                    st_eng.dma_start(
                        out=yt[:, b0 : b0 + 2 * cw], in_=ysb01[:, 0 : 2 * cw]
                    )
                seg_base += seg
    nc.compile()
    return nc


def _plan(counts):
    """Snake-assign experts to cores and derive shared segment sizes.

    Returns (groups, segs): groups[c] lists core c's experts largest-first;
    segs[j] = max over cores of the j-th largest count, rounded to SEG_GRAN.
    """
    order = np.argsort(-counts, kind="stable")
    groups = [[] for _ in range(NCORES)]
    for r, e in enumerate(order):
        band, pos = divmod(r, NCORES)
        c = pos if band % 2 == 0 else NCORES - 1 - pos
        groups[c].append(int(e))
    for g in groups:
        g.sort(key=lambda e: -counts[e])
    segs = tuple(
        int(-(-max(int(counts[g[j]]) for g in groups) // SEG_GRAN)) * SEG_GRAN
        for j in range(EPC)
    )
    return groups, segs


def kernel(inp, weight, fwd_expert_count, capacity):
    global last_exec_time_ns, last_results

    cfg = _Cfg()
    inp = np.asarray(inp)
    weight = np.asarray(weight)
    counts = np.asarray(fwd_expert_count).astype(np.int64)
    T, d_in = inp.shape
    E = weight.shape[0]
    assert d_in == D and E == NCORES * EPC
    assert int(counts.sum()) == T, "counts must cover all tokens"

    ends = np.cumsum(counts)
    starts = ends - counts
    groups, segs = _plan(counts)
    seg_off = [0]
    for s in segs:
        seg_off.append(seg_off[-1] + s)
    width = seg_off[-1]
    wrow = EPC * D * cfg.np_w.itemsize
    wpad = 2 * wrow

    # host-side scatter: transpose once, then contiguous row-slice copies
    xt_full = np.ascontiguousarray(inp.T)  # [D, T] float32
    if cfg.xscale != 1.0:
        xt_full = xt_full * np.float32(cfg.xscale)
    if cfg.np_x != np.float32:
        if cfg.np_x.itemsize == 1:
            xt_full = np.clip(xt_full, -15.5, 15.5)
        xt_full = xt_full.astype(cfg.np_x)

    in_maps = []
    for dcore in range(NCORES):
        # per-segment padded panel [D, width] in the old orientation
        xo = np.zeros((D, width), dtype=cfg.np_x)
        for j in range(EPC):
            e = groups[dcore][j]
            s, c = int(starts[e]), int(counts[e])
            xo[:, seg_off[j] : seg_off[j] + c] = xt_full[:, s : s + c]
        # chunk-major device layout [128, wpad + 2*width], w bytes first
        xd = np.empty((128, wpad + 2 * width), dtype=cfg.np_x)
        wl = weight[groups[dcore]]  # [EPC, out, in] in segment order
        wt = np.ascontiguousarray(wl.transpose(2, 0, 1).reshape(D, EPC * D))
        if cfg.xscale != 1.0:
            # x ships as xscale*x; fold 1/xscale into w so PSUM holds
            # unscaled y (e3m4 y-cast must stay within +-15.5)
            wt = wt * np.float32(1.0 / cfg.xscale)
        wb = wt.astype(cfg.np_w).view(np.uint8)  # [256, wrow]
        xdb = xd.view(np.uint8)
        wblk = wpad // EPC
        for j in range(EPC):
            eb = j * wrow // EPC
            xdb[:, j * wblk : j * wblk + wblk // 2] = wb[0:128, eb : eb + wblk // 2]
            xdb[:, j * wblk + wblk // 2 : (j + 1) * wblk] = wb[
                128:256, eb : eb + wblk // 2
            ]
        for j in range(EPC):
            for coff, cw in _chunk_offsets(
                segs[j], cfg.chunk, first_split=(j == 0), last_split=(j == EPC - 1)
            ):
                b0 = wpad + 2 * (seg_off[j] + coff)
                t0 = seg_off[j] + coff
                blk = xd[:, b0 : b0 + 2 * cw].reshape(128, 2, cw)
                blk[:, 0, :] = xo[0:128, t0 : t0 + cw]
                blk[:, 1, :] = xo[128:256, t0 : t0 + cw]
        in_maps.append({"xt": xd})

    key = cfg.key(segs)
    if key not in _prog_cache:
        _prog_cache[key] = _build_program(cfg, segs)
    nc = _prog_cache[key]

    trace = bool(int(os.environ.get("BASSMOE_TRACE", "0")))
    res = run_bass_kernel_spmd(nc, in_maps, list(range(NCORES)), trace=trace)
    last_exec_time_ns = res.exec_time_ns
    last_results = res

    # gather back to token order (y is unscaled: 1/xscale is folded into w)
    out_t = np.empty((D, T), dtype=np.float32)
    for dcore in range(NCORES):
        yd = np.asarray(res.results[dcore]["yt"]).astype(np.float32)
        for j in range(EPC):
            e = groups[dcore][j]
            s, c = int(starts[e]), int(counts[e])
            done = 0
            for coff, cw in _chunk_offsets(
                segs[j], cfg.chunk, first_split=(j == 0), last_split=(j == EPC - 1)
            ):
                if done >= c:
                    break
                take = min(cw, c - done)
                b0 = 2 * (seg_off[j] + coff)
                blk = yd[:, b0 : b0 + 2 * cw].reshape(128, 2, cw)
                out_t[0:128, s + done : s + done + take] = blk[:, 0, :take]
                out_t[128:256, s + done : s + done + take] = blk[:, 1, :take]
                done += take
            assert done >= c
    return np.ascontiguousarray(out_t.T)


# revision 62
# speedup vs baseline: 1.0460x; 1.0011x over previous
"""MoE grouped-GEMM (FMoELinear) on 8 trn2 NeuronCores.

Strategy (expert parallelism):
  - 32 experts, 8 cores -> 4 experts per core, chosen by a snake assignment
    over the count-sorted experts so per-core token totals balance.
  - Tokens arrive pre-sorted by expert. Each core's 4 experts map to 4
    fixed-size SEGMENTS (sizes shared by all cores so one SPMD program
    serves every core): segment j holds the core's j-th largest expert,
    and S_j = max over cores of the j-th largest count, rounded to 32.
    This cuts padding from a uniform per-expert cap to ~0.3%.
  - Device computes yt[o, t] = sum_i W[e][o, i] * x[t, i] per segment with
    the weight stationary in the PE array:
        lhsT = wt[i_chunk, j*256 + oc*128 : +128]   (128 x 128, stationary)
        rhs  = xt tile    [i_chunk, token span]     (128 x <=512, moving)
    accumulating the two i-chunks into PSUM, then casting PSUM->SBUF->HBM.
  - Host gathers the non-padded columns back into token order.

Precision/bandwidth plan: rel-err budget is 2e-2; stream x and y as fp8
E3M4 (~1.3% rms quant noise each; measured total 1.75e-2) to halve HBM
traffic and SBUF pressure. x is pre-scaled by XSCALE (folded back via
w/XSCALE) so fewer values land in the E3M4 subnormal range while PSUM y
stays unscaled (|y|max ~8.9 must fit E3M4's +-15.5 on the cast).

Engine layout: PE does 2 matmuls per token span (K=256 split in two
128-row chunks) at 1 cycle/column; the PSUM->SBUF casts alternate between
DVE and ACT. Only three DMA rings exist (sync/scalar HWDGE + gpsimd
SWDGE) at ~165-200 GB/s each, and engine instruction streams are
in-order, so x loads ride the sync ring (which does nothing else), the
weight panels and half the y stores ride the scalar ring, and the other
half of the stores ride the gpsimd ring. The first chunk is split into
512-token pieces and one early chunk is loaded via gpsimd so supply
outruns the PE during queue spin-up; warm-up matmuls cover that window
and pull the PE out of its low power-state. The final chunks store in
512-token pieces alternating scalar/sync rings, anchored so the very
last piece casts on both DVE and ACT in parallel and stores on the
then-idle sync ring (the gpsimd ring drains slowly at teardown).

DRAM layout: x and y are stored chunk-major as [128, sum_j 2*S_j] so every
full-chunk DMA is a single 2*CHUNK-byte contiguous run per partition
(bigger SDMA descriptors -> better per-queue DMA throughput). Block c at
token offset t0 holds tokens [t0, t0+cw) as [2, cw]: row 0 = in-features
0..127, row 1 = in-features 128..255 (for y: out-features).
"""

import os
import sys
import types

import ml_dtypes
import numpy as np

import concourse.bacc as bacc
import concourse.mybir as mybir
import concourse.tile as tile
from concourse.bass_utils import run_bass_kernel_spmd


def _ensure_axon_hooks_importable():
    """bass_utils imports antenv.axon_hooks when tracing is requested; some
    images lack that module. Provide a no-op fallback so a stray BASS_TRACE
    env var can't crash the kernel (tracing then degrades gracefully)."""
    try:
        import antenv  # noqa: F401
    except ImportError:
        return
    try:
        import antenv.axon_hooks  # noqa: F401
    except ImportError:
        mod = types.ModuleType("antenv.axon_hooks")
        holder = [None]
        mod.set_axon_ntff_profile_hook = lambda h: holder.__setitem__(0, h)
        mod.get_axon_ntff_profile_hook = lambda: holder[0]
        sys.modules["antenv.axon_hooks"] = mod
        import antenv as _antenv

        _antenv.axon_hooks = mod


_ensure_axon_hooks_importable()

NCORES = 8
D = 256  # in/out feature dim
EPC = 4  # experts per core
SEG_GRAN = 32  # segment size granularity (tokens)

# observability for test harness
last_exec_time_ns = None
last_results = None

_prog_cache = {}


def _dt1(name):
    if name == "f32":
        return mybir.dt.float32, np.dtype(np.float32)
    if name == "f32r":
        return mybir.dt.float32r, np.dtype(np.float32)
    if name == "f16":
        return mybir.dt.float16, np.dtype(np.float16)
    if name == "bf16":
        return mybir.dt.bfloat16, np.dtype(ml_dtypes.bfloat16)
    if name == "f8e3":
        return mybir.dt.float8e3, np.dtype(ml_dtypes.float8_e3m4)
    if name == "f8e4":
        return mybir.dt.float8e4, np.dtype(ml_dtypes.float8_e4m3)
    if name == "f8e5":
        return mybir.dt.float8e5, np.dtype(ml_dtypes.float8_e5m2)
    raise ValueError(name)


class _Cfg:
    def __init__(self):
        # "xdt" or "xdt+wdt": moving (x) and stationary (w) matmul dtypes
        self.mm_dt = os.environ.get("BASSMOE_MM_DT", "f8e3+f16")
        self.y_dt = os.environ.get("BASSMOE_Y_DT", "f8e3")
        self.xscale = float(os.environ.get("BASSMOE_XSCALE", "2"))
        self.chunk = int(os.environ.get("BASSMOE_CHUNK", "2048"))
        # cast-engine pattern, cycled per PSUM-bank cast: d=DVE, a=ACT
        self.cast_pat = os.environ.get("BASSMOE_CAST_PAT", "da")
        # engines issuing y stores, cycled per chunk: g=gpsimd, a=ACT, s=SP
        self.st_pat = os.environ.get("BASSMOE_ST_PAT", "ga")
        self.xbufs = int(os.environ.get("BASSMOE_XBUFS", "8"))
        self.ybufs = int(os.environ.get("BASSMOE_YBUFS", "6"))
        self.psbufs = int(os.environ.get("BASSMOE_PSBUFS", "8"))
        self.wide_cast = bool(int(os.environ.get("BASSMOE_WIDE_CAST", "0")))
        self.warm_mms = int(os.environ.get("BASSMOE_WARM_MMS", "6"))
        self.warm_inter = int(os.environ.get("BASSMOE_WARM_INTER", "0"))
        parts = self.mm_dt.split("+")
        self.dt_x, self.np_x = _dt1(parts[0])
        self.dt_w, self.np_w = _dt1(parts[-1])
        self.dt_y, self.np_y = _dt1(self.y_dt)

    def key(self, segs):
        return (
            segs,
            self.mm_dt,
            self.y_dt,
            self.chunk,
            self.cast_pat,
            self.st_pat,
            self.xbufs,
            self.ybufs,
            self.psbufs,
            self.warm_mms,
            self.warm_inter,
            self.wide_cast,
        )


def _chunk_offsets(
    seg: int, chunk: int, first_split: bool = False, last_split: bool = False
):
    """(offset, width) chunks covering [0, seg), width <= chunk.

    first_split breaks the leading chunk into small pieces so the very
    first matmul can start as soon as a short prefix of x has landed;
    last_split tapers the trailing chunks the same way so the final store
    transfers (which serialize after the last casts) are short."""
    out = []
    off = 0
    while off < seg:
        w = min(chunk, seg - off)
        # absorb a sub-512 trailing remainder into the preceding chunk so
        # no degenerate mini-chunk pays its own DMA/cast/store overhead
        if (
            not (first_split and off == 0)
            and not (last_split and off + 2 * chunk >= seg)
            and 0 < seg - (off + w) < 512
        ):
            w = seg - off
        if first_split and off == 0:
            # two 1024-token pieces: small enough that the first matmul
            # starts early, few enough that the serial ~0.7us DMA-trigger
            # instructions don't delay the following chunk loads
            s = 0
            while s < w:
                out.append((off + s, min(1024, w - s)))
                s += 1024
        elif last_split and off + 2 * chunk >= seg:
            s = 0
            while s < w:
                r = w - s
                if off + w == seg and r <= 512:
                    # the very last piece: split small so the final
                    # cast->store chain after the last matmul is short
                    if r > 256:
                        out.append((off + s, r - 192))
                        s += r - 192
                        r = 192
                    out.append((off + s, r))
                    s += r
                else:
                    out.append((off + s, min(512, r)))
                    s += 512
        else:
            out.append((off, w))
        off += w
    return out


def _splits(width: int):
    """(offset, width) matmul spans <= 512 covering [0, width)."""
    out = []
    off = 0
    while off < width:
        w = min(512, width - off)
        out.append((off, w))
        off += w
    return out


def _build_program(cfg: _Cfg, segs: tuple):
    """Build the SPMD Bass program for per-segment token sizes `segs`."""
    width = sum(segs)
    CHUNK = cfg.chunk
    # widest chunk across all segments (remainder-absorbing chunks can
    # exceed CHUNK by up to 511 tokens) sizes the x/y tile pools
    maxw = max(
        cw
        for j, seg in enumerate(segs)
        for _, cw in _chunk_offsets(
            seg, CHUNK, first_split=(j == 0), last_split=(j == len(segs) - 1)
        )
    )

    nc = bacc.Bacc(
        "TRN2",
        target_bir_lowering=False,
        debug=False,
        enable_asserts=False,
        num_devices=NCORES,
    )
    # chunk-major layout: [128, wpad + 2*width]; the first wpad columns hold
    # the raw bytes of the four weight panels so they ride ahead of the
    # token stream (bitcast back to the weight dtype in SBUF)
    wrow = EPC * D * cfg.np_w.itemsize  # bytes per partition per panel
    wpad = 2 * wrow
    xt = nc.dram_tensor(
        "xt", [128, wpad + 2 * width], cfg.dt_x, kind="ExternalInput"
    ).ap()
    yt = nc.dram_tensor("yt", [128, 2 * width], cfg.dt_y, kind="ExternalOutput").ap()

    cast_engs = [{"d": nc.vector, "a": nc.scalar}[c] for c in cfg.cast_pat]
    st_engs = [
        {"g": nc.gpsimd, "a": nc.scalar, "s": nc.sync}[c] for c in cfg.st_pat
    ]

    with tile.TileContext(nc) as tc:
        with (
            tc.tile_pool(name="w", bufs=1) as wpool,
            tc.tile_pool(name="x", bufs=cfg.xbufs) as xpool,
            tc.tile_pool(name="y", bufs=cfg.ybufs) as ypool,
            tc.tile_pool(name="ps", bufs=cfg.psbufs, space="PSUM") as pspool,
        ):
            # stationary weights, shipped as raw bytes at the head of xt,
            # packed per segment: [w0_j | w1_j] blocks of wblk bytes.
            # Segment 0's block loads first on the scalar ring (in parallel
            # with the first token pieces on the sync ring) so the first
            # real matmul waits only ~0.8us of weight bytes.
            wblk = wpad // EPC  # bytes per segment (both K-halves)
            w01 = wpool.tile([128, wpad], cfg.dt_x, tag="w01")
            nc.scalar.dma_start(out=w01[:, 0:wblk], in_=xt[:, 0:wblk])
            nc.scalar.dma_start(out=w01[:, wblk:wpad], in_=xt[:, wblk:wpad])
            qb = wblk // 4  # bytes per [128,128] weight tile
            wap = [
                [
                    [
                        w01[:, j * wblk + h * 2 * qb + oc * qb :][:, 0:qb].bitcast(
                            cfg.dt_w
                        )
                        for oc in range(2)
                    ]
                    for h in range(2)
                ]
                for j in range(EPC)
            ]

            # dummy matmuls during the DMA-warmup window pull the PE out of
            # its low p-state before the first real chunk lands
            if cfg.warm_mms:
                wdum = wpool.tile([128, 16], cfg.dt_w, tag="wdum")
                xdum = wpool.tile([128, 512], cfg.dt_x, tag="xdum")
                nc.gpsimd.memset(wdum[:], 0)
                nc.gpsimd.memset(xdum[:], 0)
                for _ in range(cfg.warm_mms):
                    ps = pspool.tile([128, 512], mybir.dt.float32, tag="ps")
                    nc.tensor.matmul(
                        ps[0:16, :], wdum[:], xdum[:], start=True, stop=True
                    )

            # x loads ride the sync ring, which does nothing else so its
            # DMA triggers never queue behind semaphore-waiting work (engine
            # streams are in-order). One early chunk rides the gpsimd ring
            # (its trigger precedes any store in that stream) so the supply
            # builds a cushion while the PE is still on warm-up matmuls.
            def _ld_eng(i):
                return nc.gpsimd if i == 2 else nc.sync

            castidx = 0
            chidx = 0
            ldidx = 0
            seg_base = 0
            nseg = len(segs)
            for j in range(nseg):
                seg = segs[j]
                chunks_j = _chunk_offsets(
                    seg, CHUNK, first_split=(j == 0), last_split=(j == nseg - 1)
                )
                for k, (coff, cw) in enumerate(chunks_j):
                    rev = len(chunks_j) - 1 - k if j == nseg - 1 else -1
                    bx = wpad + 2 * (seg_base + coff)  # xt block offset
                    b0 = 2 * (seg_base + coff)  # yt block offset
                    # tiles hold the piece contiguously ([2, cw] halves
                    # back-to-back) so every DMA is a single contiguous run
                    # per partition regardless of piece width
                    x01 = xpool.tile([128, 2 * maxw], cfg.dt_x, tag="x01")
                    ld_eng = _ld_eng(ldidx)
                    ldidx += 1
                    ld_eng.dma_start(
                        out=x01[:, 0 : 2 * cw], in_=xt[:, bx : bx + 2 * cw]
                    )
                    x0 = x01[:, 0:cw]
                    x1 = x01[:, cw : 2 * cw]
                    # the final chunk's pieces alternate their stores over
                    # the two HWDGE rings so the post-compute drain is short
                    # (the gpsimd SWDGE ring drains slowly at teardown)
                    is_tail = j == nseg - 1 and coff + 2 * CHUNK >= seg
                    ysb01 = ypool.tile([128, 2 * maxw], cfg.dt_y, tag="y01")

                    def _mm_pair(ps_dst, oc, soff, sw):
                        nc.tensor.matmul(
                            ps_dst,
                            wap[j][0][oc],
                            x0[:, soff : soff + sw],
                            start=True,
                            stop=False,
                        )
                        nc.tensor.matmul(
                            ps_dst,
                            wap[j][1][oc],
                            x1[:, soff : soff + sw],
                            start=False,
                            stop=True,
                        )

                    def _cast(dst, src, eng=None):
                        nonlocal castidx
                        if eng is None:
                            eng = cast_engs[castidx % len(cast_engs)]
                        castidx += 1
                        if eng is nc.scalar:
                            eng.copy(dst, src)
                        else:
                            eng.tensor_copy(dst, src)

                    if cfg.wide_cast and cw <= 512:
                        # small piece: both oc halves accumulate into the two
                        # banks of one wide PSUM tile; a single cast drains it
                        ps = pspool.tile([128, 1024], mybir.dt.float32, tag="ps")
                        for oc in range(2):
                            _mm_pair(ps[:, oc * 512 : oc * 512 + cw], oc, 0, cw)
                        if cw == 512:
                            _cast(ysb01[:, 0 : 2 * cw], ps[:])
                        else:
                            _cast(
                                ysb01[:, 0 : 2 * cw].rearrange(
                                    "p (b c) -> p b c", b=2
                                ),
                                ps[:].rearrange("p (b c) -> p b c", b=2)[
                                    :, :, 0:cw
                                ],
                            )
                    elif cfg.wide_cast:
                        # big chunk: adjacent 512-spans pair into one wide
                        # PSUM tile (contiguous columns), one cast per pair
                        for oc in range(2):
                            ysb = ysb01[:, oc * cw : (oc + 1) * cw]
                            spans = _splits(cw)
                            i = 0
                            while i < len(spans):
                                s0, w0 = spans[i]
                                ps = pspool.tile(
                                    [128, 1024], mybir.dt.float32, tag="ps"
                                )
                                _mm_pair(ps[:, 0:w0], oc, s0, w0)
                                tot = w0
                                i += 1
                                if w0 == 512 and i < len(spans):
                                    s1, w1 = spans[i]
                                    _mm_pair(ps[:, 512 : 512 + w1], oc, s1, w1)
                                    tot += w1
                                    i += 1
                                _cast(ysb[:, s0 : s0 + tot], ps[:, 0:tot])
                    else:
                        for oc in range(2):
                            ysb = ysb01[:, oc * cw : (oc + 1) * cw]
                            for soff, sw in _splits(cw):
                                ps = pspool.tile(
                                    [128, 512], mybir.dt.float32, tag="ps"
                                )
                                _mm_pair(ps[:, :sw], oc, soff, sw)
                                # the very last piece casts its halves on
                                # both engines in parallel so the final
                                # store isn't stuck behind one engine
                                feng = None
                                if rev == 0:
                                    feng = nc.scalar if oc == 0 else nc.vector
                                _cast(ysb[:, soff : soff + sw], ps[:, :sw], feng)
                    # single store per chunk (both oc halves); alternate
                    # rings so no single DMA queue limits the drain. Tail
                    # stores alternate the two HWDGE rings anchored so the
                    # FINAL piece rides the idle sync ring.
                    if is_tail:
                        st_eng = nc.scalar if rev % 2 == 1 else nc.sync
                    else:
                        st_eng = st_engs[chidx % len(st_engs)]
                    chidx += 1
                    st_eng.dma_start(
                        out=yt[:, b0 : b0 + 2 * cw], in_=ysb01[:, 0 : 2 * cw]
                    )
                seg_base += seg
    nc.compile()
    return nc


def _plan(counts):
    """Snake-assign experts to cores and derive shared segment sizes.

    Returns (groups, segs): groups[c] lists core c's experts largest-first;
    segs[j] = max over cores of the j-th largest count, rounded to SEG_GRAN.
    """
    order = np.argsort(-counts, kind="stable")
    groups = [[] for _ in range(NCORES)]
    for r, e in enumerate(order):
        band, pos = divmod(r, NCORES)
        c = pos if band % 2 == 0 else NCORES - 1 - pos
        groups[c].append(int(e))
    for g in groups:
        g.sort(key=lambda e: -counts[e])
    segs = tuple(
        int(-(-max(int(counts[g[j]]) for g in groups) // SEG_GRAN)) * SEG_GRAN
        for j in range(EPC)
    )
    return groups, segs


def kernel(inp, weight, fwd_expert_count, capacity):
    global last_exec_time_ns, last_results

    cfg = _Cfg()
    inp = np.asarray(inp)
    weight = np.asarray(weight)
    counts = np.asarray(fwd_expert_count).astype(np.int64)
    T, d_in = inp.shape
    E = weight.shape[0]
    assert d_in == D and E == NCORES * EPC
    assert int(counts.sum()) == T, "counts must cover all tokens"

    ends = np.cumsum(counts)
    starts = ends - counts
    groups, segs = _plan(counts)
    seg_off = [0]
    for s in segs:
        seg_off.append(seg_off[-1] + s)
    width = seg_off[-1]
    wrow = EPC * D * cfg.np_w.itemsize
    wpad = 2 * wrow

    # host-side scatter: transpose once, then contiguous row-slice copies
    xt_full = np.ascontiguousarray(inp.T)  # [D, T] float32
    if cfg.xscale != 1.0:
        xt_full = xt_full * np.float32(cfg.xscale)
    if cfg.np_x != np.float32:
        if cfg.np_x.itemsize == 1:
            xt_full = np.clip(xt_full, -15.5, 15.5)
        xt_full = xt_full.astype(cfg.np_x)

    in_maps = []
    for dcore in range(NCORES):
        # per-segment padded panel [D, width] in the old orientation
        xo = np.zeros((D, width), dtype=cfg.np_x)
        for j in range(EPC):
            e = groups[dcore][j]
            s, c = int(starts[e]), int(counts[e])
            xo[:, seg_off[j] : seg_off[j] + c] = xt_full[:, s : s + c]
        # chunk-major device layout [128, wpad + 2*width], w bytes first
        xd = np.empty((128, wpad + 2 * width), dtype=cfg.np_x)
        wl = weight[groups[dcore]]  # [EPC, out, in] in segment order
        wt = np.ascontiguousarray(wl.transpose(2, 0, 1).reshape(D, EPC * D))
        if cfg.xscale != 1.0:
            # x ships as xscale*x; fold 1/xscale into w so PSUM holds
            # unscaled y (e3m4 y-cast must stay within +-15.5)
            wt = wt * np.float32(1.0 / cfg.xscale)
        wb = wt.astype(cfg.np_w).view(np.uint8)  # [256, wrow]
        xdb = xd.view(np.uint8)
        wblk = wpad // EPC
        for j in range(EPC):
            eb = j * wrow // EPC
            xdb[:, j * wblk : j * wblk + wblk // 2] = wb[0:128, eb : eb + wblk // 2]
            xdb[:, j * wblk + wblk // 2 : (j + 1) * wblk] = wb[
                128:256, eb : eb + wblk // 2
            ]
        for j in range(EPC):
            for coff, cw in _chunk_offsets(
                segs[j], cfg.chunk, first_split=(j == 0), last_split=(j == EPC - 1)
            ):
                b0 = wpad + 2 * (seg_off[j] + coff)
                t0 = seg_off[j] + coff
                blk = xd[:, b0 : b0 + 2 * cw].reshape(128, 2, cw)
                blk[:, 0, :] = xo[0:128, t0 : t0 + cw]
                blk[:, 1, :] = xo[128:256, t0 : t0 + cw]
        in_maps.append({"xt": xd})

    key = cfg.key(segs)
    if key not in _prog_cache:
        _prog_cache[key] = _build_program(cfg, segs)
    nc = _prog_cache[key]

    trace = bool(int(os.environ.get("BASSMOE_TRACE", "0")))
    res = run_bass_kernel_spmd(nc, in_maps, list(range(NCORES)), trace=trace)
    last_exec_time_ns = res.exec_time_ns
    last_results = res

    # gather back to token order (y is unscaled: 1/xscale is folded into w)
    out_t = np.empty((D, T), dtype=np.float32)
    for dcore in range(NCORES):
        yd = np.asarray(res.results[dcore]["yt"]).astype(np.float32)
        for j in range(EPC):
            e = groups[dcore][j]
            s, c = int(starts[e]), int(counts[e])
            done = 0
            for coff, cw in _chunk_offsets(
                segs[j], cfg.chunk, first_split=(j == 0), last_split=(j == EPC - 1)
            ):
                if done >= c:
                    break
                take = min(cw, c - done)
                b0 = 2 * (seg_off[j] + coff)
                blk = yd[:, b0 : b0 + 2 * cw].reshape(128, 2, cw)
                out_t[0:128, s + done : s + done + take] = blk[:, 0, :take]
                out_t[128:256, s + done : s + done + take] = blk[:, 1, :take]
                done += take
            assert done >= c
    return np.ascontiguousarray(out_t.T)
